# revision 26
# baseline (speedup 1.0000x reference)
"""Sliding-window GQA causal self-attention block for 8 trn2 NeuronCores.

Sharding: batch (4) x T-halves (2) -> 8 cores, no collectives. Each core gets
x.T for its T-half plus a 256-row key/value halo and computes its (1024, 1024)
slice of the output.

v2 design (cost-model driven):
- All attention matmuls pack the 4 heads of a kv group along the moving dim
  (N=512) so PE engine time dominates instruction/queue overheads.
- Scores per (q-block, group): 3 key blocks; exp on ACT (scale=1/8, no max
  subtraction); band-edge masking via Pool affine_select (fill=0) directly on
  the exp'd tile; halo-padding masking via a per-core data mask multiplied on
  DVE (broadcast along free dims).
- att@v uses a ones column appended to v so the softmax denominator falls out
  of the same matmul; reciprocal on DVE; one normalize mul per partition-half
  writes straight into the output-projection layout (yTn).
- Output projection per 128-token tile -> bf16 SBUF staging -> single DMA.
"""

import dataclasses

import numpy as np
import ml_dtypes

import concourse.bass as bass
import concourse.mybir as mybir
import concourse.tile as tile
from concourse import bacc
from concourse.bass_utils import run_bass_kernel_spmd

BF = ml_dtypes.bfloat16
F32 = mybir.dt.float32
BF16 = mybir.dt.bfloat16

B, T, C = 4, 2048, 1024
H, KV, HD = 16, 4, 64
WIN = 256
TL = T // 2            # 1024 own rows per core
TH = TL + WIN          # 1280 with halo
NQB = TL // 128        # 8 query blocks
NKB = TH // 128        # 10 key blocks


def _bcast_ap(ap_view, dims):
    """Replace an AP's dims: list of [step, num]; first entry is partitions."""
    return dataclasses.replace(ap_view, ap=dims)


def _build_program():
    nc = bacc.Bacc("TRN2", target_bir_lowering=False, debug=False, num_devices=8)
    dt = mybir.dt
    xT = nc.dram_tensor("xT", [2, C, TH], dt.float8e4,
                        kind="ExternalInput").ap()
    wqk = nc.dram_tensor("wqk", [128, 2, 8, 1280], dt.float8e4,
                         kind="ExternalInput").ap()
    wv = nc.dram_tensor("wv", [128, 2, 8, 256], dt.float8e4,
                        kind="ExternalInput").ap()
    wp = nc.dram_tensor("wp", [128, 8, C], dt.bfloat16, kind="ExternalInput").ap()
    cq = nc.dram_tensor("cq", [2, 128, TL], dt.bfloat16, kind="ExternalInput").ap()
    ck = nc.dram_tensor("ck", [2, 128, TH], dt.bfloat16, kind="ExternalInput").ap()
    padv = nc.dram_tensor("padv", [128, 2, 256], dt.bfloat16,
                          kind="ExternalInput").ap()
    tri = nc.dram_tensor("tri", [128, 2, 128], dt.bfloat16,
                         kind="ExternalInput").ap()
    out = nc.dram_tensor("out", [TL, C], dt.bfloat16,
                         kind="ExternalOutput").ap()

    with tile.TileContext(nc) as tc:
        _kernel_body(tc, nc, xT, wqk, wv, wp, cq, ck, padv, tri, out)
    nc.compile()
    return nc


def _kernel_body(tc, nc, xT, wqk, wv, wp, cq, ck, padv, tri, out):
    import contextlib
    ctx = contextlib.ExitStack()
    fExp = mybir.ActivationFunctionType.Exp
    with ctx:
        persist = ctx.enter_context(tc.tile_pool(name="persist", bufs=1))

        # ---- persistent inputs (interleaved per-k-tile loads) ----
        F8 = mybir.dt.float8e4
        x_sb = persist.tile([128, 2, 8, TH], F8, tag="x_sb", name="x_sb")
        wqk_sb = persist.tile([128, 2, 8, 1280], F8, tag="wqk_sb",
                              name="wqk_sb")
        wv_sb = persist.tile([128, 2, 8, 256], F8, tag="wv_sb", name="wv_sb")
        wp_sb = persist.tile([128, 8, C], BF16, tag="wp_sb", name="wp_sb")
        nc.sync.dma_start(out=wv_sb[:], in_=wv)
        for kc in range(8):
            nc.sync.dma_start(
                out=x_sb[:, :, kc, :],
                in_=xT[:, kc * 128:(kc + 1) * 128, :].rearrange(
                    "s p t -> p s t"))
        for kc in range(8):
            nc.sync.dma_start(out=wqk_sb[:, :, kc, :], in_=wqk[:, :, kc, :])
        cq_sb = persist.tile([128, 2, TL], BF16, tag="cq_sb", name="cq_sb")
        ck_sb = persist.tile([128, 2, TH], BF16, tag="ck_sb", name="ck_sb")
        nc.sync.dma_start(out=cq_sb[:], in_=cq.rearrange("s p t -> p s t"))
        nc.sync.dma_start(out=ck_sb[:], in_=ck.rearrange("s p t -> p s t"))
        tri_sb = persist.tile([128, 2, 128], BF16, tag="tri_sb", name="tri_sb")
        nc.sync.dma_start(out=tri_sb[:], in_=tri)

        # persistent compute tensors
        # qT[g]: [64 dims (e/o interleaved), head, tok]
        # kT: [64 dims (e/o interleaved), group, tok(halo)]
        qT = [persist.tile([64, 4, TL], BF16, tag=f"qT{g}",
                           name=f"qT{g}") for g in range(KV)]
        kT = persist.tile([64, KV, TH], BF16, tag="kT", name="kT")
        v128 = persist.tile([128, NKB, KV, 128], BF16, tag="v128", name="v128")
        yTn = persist.tile([128, 8, TL], BF16, tag="yTn", name="yTn")
        # qraw/keo: rope outputs; dim1 = even/odd slot
        qraw = persist.tile([128, 2, KV, TL], BF16, tag="qraw", name="qraw")
        keo = persist.tile([128, 2, TH], BF16, tag="keo", name="keo")
        # halo-validity column (0/1) for key blocks 0,1 -> denominator mask
        nc.sync.dma_start(
            out=v128[:, 0:2, :, 64:128],
            in_=padv.rearrange("p b (g c) -> p b g c", c=64))

        ppA = ctx.enter_context(
            tc.tile_pool(name="ppA", bufs=2, space="PSUM"))     # [128,1024]
        ppB = ctx.enter_context(
            tc.tile_pool(name="ppB", bufs=4, space="PSUM"))     # [128,512]
        ropes = ctx.enter_context(tc.tile_pool(name="ropes", bufs=2))
        atts = ctx.enter_context(tc.tile_pool(name="atts", bufs=6))
        rsbs = ctx.enter_context(tc.tile_pool(name="rsbs", bufs=2))
        osbs = ctx.enter_context(tc.tile_pool(name="osbs", bufs=2))

        def rope_pair(pe, po, cs, n, oute, outo):
            """pe/po: psum [128, n]; cs: [128, 2, n] slices; out: sbuf [128,n]."""
            e_sb = ropes.tile([128, 512], BF16, tag="e_sb", name="e_sb")[:, 0:n]
            o_sb = ropes.tile([128, 512], BF16, tag="o_sb", name="o_sb")[:, 0:n]
            t1 = ropes.tile([128, 512], BF16, tag="t1", name="t1")[:, 0:n]
            t2 = ropes.tile([128, 512], BF16, tag="t2", name="t2")[:, 0:n]
            nc.scalar.copy(e_sb, pe)
            nc.scalar.copy(o_sb, po)
            nc.vector.tensor_mul(t1, e_sb, cs[:, 0])
            nc.vector.tensor_mul(t2, o_sb, cs[:, 1])
            nc.vector.tensor_sub(oute, t1, t2)
            nc.vector.tensor_mul(t1, e_sb, cs[:, 1])
            nc.vector.tensor_mul(t2, o_sb, cs[:, 0])
            nc.vector.tensor_add(outo, t1, t2)

        # ======== warmup: ramp the PE p-state while loads stream in ======
        warm = persist.tile([128, 512], BF16, tag="warm", name="warm")
        nc.vector.memset(warm[:], 0.0)
        wps = ppA.tile([128, 1024], F32, tag="pb", name="wps")
        for i in range(34):
            nc.tensor.matmul(wps[:, 0:128], warm[:, 0:128], warm[:, 0:128],
                             start=True, stop=True)

        # ======== phase V: v projection (natural layout + ones column) ======
        DR = mybir.MatmulPerfMode.DoubleRow
        TERMS = ((0, 0), (0, 1), (1, 0))     # (x half, w half): hh, hl, lh
        for b in range(NKB):
            pv = ppB.tile([128, 512], F32, tag="ps", name="pv")[:, 0:256]
            n = 0
            for r in range(4):
                for (xh, wh) in TERMS:
                    nc.tensor.matmul(
                        pv, x_sb[:, xh, 2 * r:2 * r + 2,
                                 b * 128:(b + 1) * 128],
                        wv_sb[:, wh, 2 * r:2 * r + 2, :],
                        start=(n == 0), stop=(n == 11), perf_mode=DR,
                        skip_group_check=True)
                    n += 1
                if b == 0 and r < 3:
                    for _ in range(14):
                        nc.tensor.matmul(wps[:, 0:128], warm[:, 0:128],
                                         warm[:, 0:128], start=True, stop=True,
                                         skip_group_check=True)
            v3 = v128[:, b]
            nc.scalar.copy(v3[:, :, 0:64],
                           pv.rearrange("p (g c) -> p g c", c=64))
            if b >= 2:
                nc.vector.memset(v3[:, :, 64:128], 1.0)

        # ======== phase K: k projection + rope + regroup ========
        for (n0, n1) in ((0, 512), (512, 1024), (1024, 1280)):
            pair = ppA.tile([128, 1024], F32, tag="pb", name="kpair")
            pe = pair[:, 0:n1 - n0]
            po = pair[:, 512:512 + n1 - n0]
            for dst, c0, c1 in ((pe, 1024, 1152), (po, 1152, 1280)):
                n = 0
                for r in range(4):
                    for (xh, wh) in TERMS:
                        nc.tensor.matmul(
                            dst, wqk_sb[:, wh, 2 * r:2 * r + 2, c0:c1],
                            x_sb[:, xh, 2 * r:2 * r + 2, n0:n1],
                            start=(n == 0), stop=(n == 11), perf_mode=DR)
                        n += 1
            rope_pair(pe, po, ck_sb[:, :, n0:n1], n1 - n0,
                      keo[:, 0, n0:n1], keo[:, 1, n0:n1])
        for g in range(KV):
            # [32 dims, 2 eo, TH] -> [64 interleaved dims, TH]
            nc.sync.dma_start(out=kT[:, g, :],
                              in_=keo[32 * g:32 * (g + 1), :, :])
        # wp load deferred off the startup critical path
        for half in range(2):
            nc.sync.dma_start(out=wp_sb[:, 4 * half:4 * (half + 1), :],
                              in_=wp[:, 4 * half:4 * (half + 1), :])

        # ======== phase Q: q projection + rope + regroup (per group) ========
        def stage_qproj(g):
            for half in range(2):
                pair = ppA.tile([128, 1024], F32, tag="pb", name="qpair")
                pe = pair[:, 0:512]
                po = pair[:, 512:1024]
                xs = x_sb[:, :, :, WIN + half * 512:WIN + (half + 1) * 512]
                for dst, c0 in ((pe, g * 128), (po, 512 + g * 128)):
                    n = 0
                    for r in range(4):
                        for (xh, wh) in TERMS:
                            nc.tensor.matmul(
                                dst,
                                wqk_sb[:, wh, 2 * r:2 * r + 2, c0:c0 + 128],
                                xs[:, xh, 2 * r:2 * r + 2, :],
                                start=(n == 0), stop=(n == 11), perf_mode=DR)
                            n += 1
                sl = slice(half * 512, (half + 1) * 512)
                rope_pair(pe, po, cq_sb[:, :, sl], 512,
                          qraw[:, 0, g, sl], qraw[:, 1, g, sl])
            for j in range(4):
                # [32 dims, 2 eo, TL] -> [64 interleaved dims, TL]
                nc.sync.dma_start(
                    out=qT[g][:, j, :],
                    in_=qraw[32 * j:32 * (j + 1), :, g, :])

        # ======== phase A: attention (qb-major) + output projection ========
        # Software-pipelined: scores/exp/mask of iteration i run ahead of
        # av/normalize of iteration i-1 so the PE never blocks on the
        # exp->mask chain.
        def stage_scores(qb, g):
            stA = ppA.tile([128, 1024], F32, tag="pb", name="stA")
            stB = ppB.tile([128, 512], F32, tag="ps", name="stB")
            qs = qT[g][:, :, qb * 128:(qb + 1) * 128]
            for cc in range(2):
                nc.tensor.matmul(
                    stA[:, cc * 512:(cc + 1) * 512],
                    kT[:, g, (qb + cc) * 128:(qb + cc + 1) * 128],
                    qs, start=True, stop=True)
            nc.tensor.matmul(stB, kT[:, g, (qb + 2) * 128:(qb + 3) * 128],
                             qs, start=True, stop=True)
            pt = atts.tile([128, 1536], BF16, tag="pt", name="pt")
            nc.scalar.activation(pt[:, 0:1024], stA[:], fExp, scale=0.125 / 4096.0)
            nc.scalar.activation(pt[:, 1024:1536], stB[:], fExp, scale=0.125 / 4096.0)
            ptv = pt[:].rearrange("p (cc j c) -> p cc j c", j=4, c=128)
            # band-edge masks, split across engines to shorten the chain:
            # cc0 (keep r > c) on Pool affine_select, cc2 (keep r <= c) on
            # DVE as a 0/1 triangle multiply (bcast over the 4 heads).
            # (halo-padding handled via the validity column in v128)
            nc.gpsimd.affine_select(
                out=ptv[:, 0], in_=ptv[:, 0],
                compare_op=mybir.AluOpType.is_ge, fill=0.0,
                base=-1, channel_multiplier=1, pattern=[[0, 4], [-1, 128]])
            t = tri_sb[:, 1, :]
            tb = _bcast_ap(t, [t.ap[0], [0, 4], [1, 128]])
            nc.vector.tensor_mul(ptv[:, 2], ptv[:, 2], tb)
            return ptv

        def stage_av(qb, g, ptv):
            yu = ppB.tile([128, 512], F32, tag="ps", name="yu")
            for n, cc in enumerate((1, 0, 2)):
                nc.tensor.matmul(yu[:], v128[:, qb + cc, g, :],
                                 ptv[:, cc], start=(n == 0), stop=(n == 2))
            # rows 64:128 of yu all hold the softmax denominator
            dnm = rsbs.tile([64, 512], F32, tag="dnm", name="dnm")
            nc.vector.reciprocal(dnm[:], yu[64:128, :])
            yuv = yu[:].rearrange("p (j c) -> p j c", c=128)
            dnv = dnm[:].rearrange("p (j c) -> p j c", c=128)
            for par in range(2):
                nc.vector.tensor_mul(
                    yTn[64 * par:64 * (par + 1), 2 * g:2 * g + 2,
                        qb * 128:(qb + 1) * 128],
                    yuv[0:64, par::2, :], dnv[:, par::2, :])

        def stage_oproj_half(qb, half):
            poh = ppB.tile([128, 512], F32, tag="ps", name="poh")
            for pr in range(8):
                nc.tensor.matmul(
                    poh[:],
                    yTn[:, pr, qb * 128:(qb + 1) * 128],
                    wp_sb[:, pr, half * 512:(half + 1) * 512],
                    start=(pr == 0), stop=(pr == 7))
            o_sb = osbs.tile([128, 512], BF16, tag="o_sb", name="o_sb")
            # 1/64 output scale folded into wp host-side
            nc.scalar.copy(o_sb[:], poh[:])
            nc.sync.dma_start(
                out=out[qb * 128:(qb + 1) * 128,
                        half * 512:(half + 1) * 512],
                in_=o_sb[:])

        # qb-major attention with a SKEW-deep software pipeline; the first
        # SKEW score stages are prefetched before the last q projection so
        # their exp/mask chains overlap qproj(g3) PE work. oproj halves are
        # deferred past av(qb, 3) to hide the normalize chain.
        SKEW = 4
        iters = [(qb, g) for qb in range(NQB) for g in range(KV)]
        for g in range(KV):
            stage_qproj(g)
        pending = []          # [(qb, g, ptv), ...]
        oproj_q = []          # (emit_at_iter, qb, half)
        for i, (qb, g) in enumerate(iters):
            ptv = stage_scores(qb, g)
            if len(pending) < SKEW:
                pending.append((qb, g, ptv))
                continue
            pqb, pg, pptv = pending.pop(0)
            stage_av(pqb, pg, pptv)
            if pg == KV - 1:
                oproj_q.append((i + 2, pqb, 0))
                oproj_q.append((i + 3, pqb, 1))
            while oproj_q and i >= oproj_q[0][0]:
                _, oqb, oh = oproj_q.pop(0)
                stage_oproj_half(oqb, oh)
            pending.append((qb, g, ptv))
        for j, (pqb, pg, pptv) in enumerate(pending):
            stage_av(pqb, pg, pptv)
            if pg == KV - 1:
                oproj_q.append((0, pqb, 0))
                oproj_q.append((0, pqb, 1))
            if j == 0 and oproj_q:
                _, oqb, oh = oproj_q.pop(0)
                stage_oproj_half(oqb, oh)
        for _, oqb, oh in oproj_q:
            stage_oproj_half(oqb, oh)


_PROGRAM_CACHE = {}


def _get_program():
    if "nc" not in _PROGRAM_CACHE:
        _PROGRAM_CACHE["nc"] = _build_program()
    return _PROGRAM_CACHE["nc"]


def prepare_in_maps(x, freqs_cos, freqs_sin, w_attn, b_attn, w_proj, b_proj):
    x = np.asarray(x, dtype=np.float32)
    freqs_cos = np.asarray(freqs_cos, dtype=np.float32)
    freqs_sin = np.asarray(freqs_sin, dtype=np.float32)
    w_attn = np.asarray(w_attn, dtype=np.float32)
    b_attn = np.asarray(b_attn, dtype=np.float32)
    w_proj = np.asarray(w_proj, dtype=np.float32)
    assert not np.any(b_attn), "kernel assumes zero qkv bias"

    # q/k channel permutation: evens block then odds block, head-major
    qch = np.arange(H * HD).reshape(H, 32, 2)
    q_perm = np.concatenate([qch[:, :, 0].reshape(-1), qch[:, :, 1].reshape(-1)])
    kch = H * HD + np.arange(KV * HD).reshape(KV, 32, 2)
    k_perm = np.concatenate([kch[:, :, 0].reshape(-1), kch[:, :, 1].reshape(-1)])
    wqk_f = w_attn[np.concatenate([q_perm, k_perm])].T * 64.0      # (1024, 1280)
    wv_f = w_attn[(H + KV) * HD:].T * 64.0                         # (1024, 256)

    F8 = ml_dtypes.float8_e4m3

    def _hilo(a):
        hi = np.clip(a, -240.0, 240.0).astype(F8)
        lo = np.clip(a - hi.astype(np.float32), -240.0, 240.0).astype(F8)
        return hi, lo

    # device layout [128, hl, kc, cols]
    wqk_hi, wqk_lo = _hilo(wqk_f)
    wqk_h = np.ascontiguousarray(
        np.stack([wqk_hi, wqk_lo], axis=1).reshape(8, 128, 2, 1280)
        .transpose(1, 2, 0, 3))
    wv_hi, wv_lo = _hilo(wv_f)
    wv_h = np.ascontiguousarray(
        np.stack([wv_hi, wv_lo], axis=1).reshape(8, 128, 2, 256)
        .transpose(1, 2, 0, 3))

    # wp rows permuted to the yTn layout: (p, pr) -> head 4*(pr//2)+2*(pr%2)
    #   + (p>=64), dim p%64;  wp[cin, pr, cout]
    p_idx = np.arange(128)[:, None]
    pr_idx = np.arange(8)[None, :]
    head = 4 * (pr_idx // 2) + 2 * (pr_idx % 2) + (p_idx >= 64)
    chan = head * 64 + (p_idx % 64)                                # (128, 8)
    # 1/64 undoes the x64 scaling of wqk/wv folded through the attention
    wp_h = np.ascontiguousarray(
        w_proj.T[chan.reshape(-1)].reshape(128, 8, C) / 64.0).astype(BF)

    # static 0/1 band-edge triangle masks: [p, 0, c] keep p > c (block cc0),
    # [p, 1, c] keep p <= c (block cc2)
    p128 = np.arange(128)
    tri = np.stack([(p128[:, None] > p128[None, :]),
                    (p128[:, None] <= p128[None, :])], axis=1)

    cos4 = np.tile(freqs_cos.T, (4, 1)).astype(np.float32)    # (128, T)
    sin4 = np.tile(freqs_sin.T, (4, 1)).astype(np.float32)

    in_maps = []
    for core in range(8):
        b, h = divmod(core, 2)
        t0 = h * TL
        xs = np.zeros((TH, C), dtype=np.float32)
        lo = max(0, t0 - WIN)
        xs[TH - (t0 + TL - lo):] = x[b, lo:t0 + TL]
        cpad = np.zeros((128, TH), dtype=np.float32)
        spad = np.zeros((128, TH), dtype=np.float32)
        cpad[:, TH - (t0 + TL - lo):] = cos4[:, lo:t0 + TL]
        spad[:, TH - (t0 + TL - lo):] = sin4[:, lo:t0 + TL]
        # halo-padding validity (0/1) per key partition for key blocks 0,1;
        # broadcast to the 4 groups x 64 "ones"-columns of v128
        r = np.arange(128)
        maskE = np.stack([(t0 - 256 + r >= 0), (t0 - 128 + r >= 0)],
                         axis=1).astype(np.float32)          # (128, 2)
        padv = np.repeat(maskE[:, :, None], 256, axis=2)     # (128, 2, 256)
        x_hi, x_lo = _hilo(np.ascontiguousarray(xs.T))
        in_maps.append({
            "xT": np.ascontiguousarray(np.stack([x_hi, x_lo])),
            "wqk": wqk_h, "wv": wv_h, "wp": wp_h,
            "cq": np.stack([cos4[:, t0:t0 + TL],
                            sin4[:, t0:t0 + TL]]).astype(BF),
            "ck": np.stack([cpad, spad]).astype(BF),
            "padv": np.ascontiguousarray(padv).astype(BF),
            "tri": np.ascontiguousarray(tri).astype(BF),
        })

    return in_maps


def kernel(**inputs):
    in_maps = prepare_in_maps(**inputs)
    nc = _get_program()
    res = run_bass_kernel_spmd(nc, in_maps, list(range(8)))
    return _gather(res, np.asarray(inputs["b_proj"], dtype=np.float32))


def _gather(res, b_proj):
    out = np.empty((B, T, C), dtype=np.float32)
    for core in range(8):
        b, h = divmod(core, 2)
        out[b, h * TL:(h + 1) * TL] = res.results[core]["out"].astype(np.float32)
    if np.any(b_proj):
        out += b_proj
    return out

